# revision 6
# baseline (speedup 1.0000x reference)
"""Trainium2 Bass kernel: NeuralNearestNeighbors continuous-KNN weight volumes.

Reference computation (per row of D.reshape(b*m, o), K=8 rounds):
    logits = D / exp(log_temp)
    for k in range(K):
        w_k = log_softmax(logits);  out_k = exp(w_k)
        logits = logits + log1mexp(w_k)          # log(1 - p_k)
    W = stack(out_k, axis=-1)                     # (b, m, o, K)

Exp-space identity: with p_k = softmax(logits_k),
    exp(logits_{k+1}) = exp(logits_k) * (1 - p_k)
so in normalized space F_k = p_k:
    F_{k+1} = (F_k - F_k^2) / (1 - sum_o F_k^2)
On device we keep a (sign-flipped) unnormalized state G and per-row scalar g
with F = G * g:
    G_0 = exp(D/T)            a_0 = sum(G_0)        g_0 = 1/a_0      (positive)
    G_{k+1} = (F_k - 1)*F_k   a_k = sum(G_{k+1}) = t_k - 1 < 0
    g_{k+1} = 1/a_k  (negative; signs cancel in F = G*g)
Each round is exactly 2 full-tile engine ops:
    pass1 (ACT):  F_k = Copy(G * g)    -> written k-strided into the out tile
    pass2 (DVE):  scalar_tensor_tensor (F-1)*F with accum_out  -> new G + a
plus a [P,1] reciprocal.

Sharding: purely rowwise data-parallel over b*m = 16384 rows; 2048 rows per
core across 8 cores; log_temp replicated.
"""

import numpy as np

B, M, O = 16, 1024, 512
K = 8
N_CORES = 8
ROWS = B * M                     # 16384
RPC = ROWS // N_CORES            # 2048 rows per core
P = 128
TILES = RPC // P                 # 16 row-tiles per core
IN_DMA_GROUP = 4                 # row-tiles per input DMA (1 MiB transfers)

_cached = None


def _build(reps=1):
    """Build and compile the Bass module (one SPMD program for all cores).

    reps>1 repeats the whole (idempotent) computation in one NEFF; used only
    for benchmarking to separate device time from dispatch overhead.
    """
    from contextlib import ExitStack

    import concourse.bacc as bacc
    import concourse.tile as tile
    from concourse import mybir

    f32 = mybir.dt.float32
    Alu = mybir.AluOpType
    Act = mybir.ActivationFunctionType

    nc = bacc.Bacc(
        "TRN2",
        target_bir_lowering=False,
        debug=False,
        enable_asserts=False,
        num_devices=N_CORES,
    )
    d = nc.dram_tensor("d", [RPC, O], f32, kind="ExternalInput").ap()
    lt = nc.dram_tensor("log_temp", [1, 1], f32, kind="ExternalInput").ap()
    w = nc.dram_tensor("w", [RPC, O * K], f32, kind="ExternalOutput").ap()

    with tile.TileContext(nc) as tc, ExitStack() as ctx:
        singles = ctx.enter_context(tc.tile_pool(name="singles", bufs=1))
        slab_pool = ctx.enter_context(tc.tile_pool(name="slab", bufs=1))
        out_pool = ctx.enter_context(tc.tile_pool(name="out", bufs=5))
        small = ctx.enter_context(tc.tile_pool(name="small", bufs=64))

        # log_temp -> 1/T = exp(-log_temp), replicated to all 128 partitions.
        lt_sb = singles.tile([P, 1], f32)
        nc.sync.dma_start(out=lt_sb[:, :], in_=lt.to_broadcast((P, 1)))
        invt = singles.tile([P, 1], f32)
        nc.scalar.activation(invt[:, :], lt_sb[:, :], Act.Exp, scale=-1.0)

        din = d.rearrange("(t p) o -> p t o", p=P)

        def body():
            # Whole per-core input slab lives in SBUF (32 KB/partition); it
            # is overwritten in place by exp() and each round's G update.
            slab = slab_pool.tile([P, TILES, O], f32)
            for gstart in range(0, TILES, IN_DMA_GROUP):
                nc.sync.dma_start(
                    out=slab[:, gstart : gstart + IN_DMA_GROUP, :],
                    in_=din[:, gstart : gstart + IN_DMA_GROUP, :],
                )
            for t in range(TILES):
                g_t = slab[:, t, :]                   # [P, O] contiguous
                out_t = out_pool.tile([P, O, K], f32)  # 16 KB/partition
                acc = small.tile([P, 1], f32)
                gam = small.tile([P, 1], f32)
                # G_0 = exp(D * 1/T), a_0 = row sums
                nc.scalar.activation(
                    g_t, g_t, Act.Exp, scale=invt[:, :], accum_out=acc[:, :]
                )
                nc.vector.reciprocal(gam[:, :], acc[:, :])
                for k in range(K):
                    f_k = out_t[:, :, k]              # stride-K view
                    nc.scalar.mul(f_k, g_t, gam[:, :])  # pass1: F = G * g
                    if k == K - 1:
                        break
                    acc = small.tile([P, 1], f32)
                    nc.vector.scalar_tensor_tensor(   # pass2: G'=(F-1)*F, a=sum
                        out=g_t,
                        in0=f_k,
                        scalar=1.0,
                        in1=f_k,
                        op0=Alu.subtract,
                        op1=Alu.mult,
                        accum_out=acc[:, :],
                    )
                    gam = small.tile([P, 1], f32)
                    nc.vector.reciprocal(gam[:, :], acc[:, :])
                nc.sync.dma_start(out=w[t * P : (t + 1) * P, :], in_=out_t[:, :, :])

        if reps > 1:
            # Benchmark mode: repeat the idempotent body in a HW loop so
            # device time can be measured by differencing two reps values.
            with tc.For_i(
                0, reps, 1,
                hint_engines=(
                    mybir.EngineType.DVE,
                    mybir.EngineType.Activation,
                    mybir.EngineType.SP,
                ),
            ):
                body()
        else:
            body()

    nc.compile()
    return nc


def _get_nc():
    global _cached
    if _cached is None:
        _cached = _build()
    return _cached


def _make_in_maps(D, log_temp):
    Dr = np.ascontiguousarray(np.asarray(D, dtype=np.float32).reshape(ROWS, O))
    lt = np.asarray(log_temp, dtype=np.float32).reshape(1, 1)
    return [
        {"d": Dr[c * RPC : (c + 1) * RPC], "log_temp": lt}
        for c in range(N_CORES)
    ]


def _gather(results):
    parts = [results[c]["w"].reshape(RPC, O, K) for c in range(N_CORES)]
    return np.concatenate(parts, axis=0).reshape(B, M, O, K)


def run_spmd(D, log_temp, trace=False, **kwargs):
    """Run on all 8 cores; returns (W, BassKernelResults)."""
    from concourse.bass_utils import run_bass_kernel_spmd

    nc = _get_nc()
    res = run_bass_kernel_spmd(
        nc, _make_in_maps(D, log_temp), list(range(N_CORES)), trace=trace, **kwargs
    )
    return _gather(res.results), res


def kernel(D, log_temp):
    W, _ = run_spmd(D, log_temp)
    return W


# revision 8
# speedup vs baseline: 2.0472x; 2.0472x over previous
"""Trainium2 Bass kernel: NeuralNearestNeighbors continuous-KNN weight volumes.

Reference computation (per row of D.reshape(b*m, o), K=8 rounds):
    logits = D / exp(log_temp)
    for k in range(K):
        w_k = log_softmax(logits);  out_k = exp(w_k)
        logits = logits + log1mexp(w_k)          # log(1 - p_k)
    W = stack(out_k, axis=-1)                     # (b, m, o, K)

Exp-space identity: with p_k = softmax(logits_k),
    exp(logits_{k+1}) = exp(logits_k) * (1 - p_k)
so in normalized space F_k = p_k:
    F_{k+1} = (F_k - F_k^2) / (1 - sum_o F_k^2)
On device we keep a (sign-flipped) unnormalized state G and per-row scalar g
with F = G * g:
    G_0 = exp(D/T)            a_0 = sum(G_0)        g_0 = 1/a_0      (positive)
    G_{k+1} = (F_k - 1)*F_k   a_k = sum(G_{k+1}) = t_k - 1 < 0
    g_{k+1} = 1/a_k  (negative; signs cancel in F = G*g)
Each round is exactly 2 full-tile engine ops:
    pass1 (ACT):  F_k = Copy(G * g)    -> written k-strided into the out tile
    pass2 (DVE):  scalar_tensor_tensor (F-1)*F with accum_out  -> new G + a
plus a [P,1] reciprocal.

Sharding: purely rowwise data-parallel over b*m = 16384 rows; 2048 rows per
core across 8 cores; log_temp replicated.
"""

import numpy as np

B, M, O = 16, 1024, 512
K = 8
N_CORES = 8
ROWS = B * M                     # 16384
RPC = ROWS // N_CORES            # 2048 rows per core
P = 128
TILES = RPC // P                 # 16 row-tiles per core
IN_DMA_GROUP = 4                 # row-tiles per input DMA (1 MiB transfers)

_cached = None


def _build(reps=1):
    """Build and compile the Bass module (one SPMD program for all cores).

    reps>1 repeats the whole (idempotent) computation in one NEFF; used only
    for benchmarking to separate device time from dispatch overhead.
    """
    from contextlib import ExitStack

    import concourse.bacc as bacc
    import concourse.tile as tile
    from concourse import mybir

    f32 = mybir.dt.float32
    Alu = mybir.AluOpType
    Act = mybir.ActivationFunctionType

    nc = bacc.Bacc(
        "TRN2",
        target_bir_lowering=False,
        debug=False,
        enable_asserts=False,
        num_devices=N_CORES,
    )
    d = nc.dram_tensor("d", [RPC, O], f32, kind="ExternalInput").ap()
    lt = nc.dram_tensor("log_temp", [1, 1], f32, kind="ExternalInput").ap()
    w = nc.dram_tensor("w", [RPC, O * K], f32, kind="ExternalOutput").ap()

    with tile.TileContext(nc) as tc, ExitStack() as ctx:
        singles = ctx.enter_context(tc.tile_pool(name="singles", bufs=1))
        slab_pool = ctx.enter_context(tc.tile_pool(name="slab", bufs=1))
        out_pool = ctx.enter_context(tc.tile_pool(name="out", bufs=5))
        small = ctx.enter_context(tc.tile_pool(name="small", bufs=64))

        # log_temp -> 1/T = exp(-log_temp), replicated to all 128 partitions.
        lt_sb = singles.tile([P, 1], f32)
        nc.sync.dma_start(out=lt_sb[:, :], in_=lt.to_broadcast((P, 1)))
        invt = singles.tile([P, 1], f32)
        nc.scalar.activation(invt[:, :], lt_sb[:, :], Act.Exp, scale=-1.0)

        din = d.rearrange("(t p) o -> p t o", p=P)

        def body():
            # Whole per-core input slab lives in SBUF (32 KB/partition); it
            # is overwritten in place by exp() and each round's G update.
            slab = slab_pool.tile([P, TILES, O], f32)
            for gstart in range(0, TILES, IN_DMA_GROUP):
                # SWDGE path: keeps both HWDGE rings free for output writes.
                nc.gpsimd.dma_start(
                    out=slab[:, gstart : gstart + IN_DMA_GROUP, :],
                    in_=din[:, gstart : gstart + IN_DMA_GROUP, :],
                )
            for t in range(TILES):
                g_t = slab[:, t, :]                   # [P, O] contiguous
                out_t = out_pool.tile([P, O, K], f32)  # 16 KB/partition
                acc = small.tile([P, 1], f32)
                gam = small.tile([P, 1], f32)
                # G_0 = exp(D * 1/T), a_0 = row sums
                nc.scalar.activation(
                    g_t, g_t, Act.Exp, scale=invt[:, :], accum_out=acc[:, :]
                )
                nc.vector.reciprocal(gam[:, :], acc[:, :])
                for k in range(K):
                    f_k = out_t[:, :, k]              # stride-K view
                    nc.scalar.mul(f_k, g_t, gam[:, :])  # pass1: F = G * g
                    if k == K - 1:
                        break
                    acc = small.tile([P, 1], f32)
                    nc.vector.scalar_tensor_tensor(   # pass2: G'=(F-1)*F, a=sum
                        out=g_t,
                        in0=f_k,
                        scalar=1.0,
                        in1=f_k,
                        op0=Alu.subtract,
                        op1=Alu.mult,
                        accum_out=acc[:, :],
                    )
                    gam = small.tile([P, 1], f32)
                    nc.vector.reciprocal(gam[:, :], acc[:, :])
                # Alternate the two HWDGE rings so output DMAs overlap.
                dma_eng = nc.sync if t % 2 == 0 else nc.scalar
                dma_eng.dma_start(out=w[t * P : (t + 1) * P, :], in_=out_t[:, :, :])

        if reps > 1:
            # Benchmark mode: repeat the idempotent body in a HW loop so
            # device time can be measured by differencing two reps values.
            with tc.For_i(
                0, reps, 1,
                hint_engines=(
                    mybir.EngineType.DVE,
                    mybir.EngineType.Activation,
                    mybir.EngineType.SP,
                ),
            ):
                body()
        else:
            body()

    nc.compile()
    return nc


def _get_nc():
    global _cached
    if _cached is None:
        _cached = _build()
    return _cached


def _make_in_maps(D, log_temp):
    Dr = np.ascontiguousarray(np.asarray(D, dtype=np.float32).reshape(ROWS, O))
    lt = np.asarray(log_temp, dtype=np.float32).reshape(1, 1)
    return [
        {"d": Dr[c * RPC : (c + 1) * RPC], "log_temp": lt}
        for c in range(N_CORES)
    ]


def _gather(results):
    parts = [results[c]["w"].reshape(RPC, O, K) for c in range(N_CORES)]
    return np.concatenate(parts, axis=0).reshape(B, M, O, K)


def run_spmd(D, log_temp, trace=False, **kwargs):
    """Run on all 8 cores; returns (W, BassKernelResults)."""
    from concourse.bass_utils import run_bass_kernel_spmd

    nc = _get_nc()
    res = run_bass_kernel_spmd(
        nc, _make_in_maps(D, log_temp), list(range(N_CORES)), trace=trace, **kwargs
    )
    return _gather(res.results), res


def kernel(D, log_temp):
    W, _ = run_spmd(D, log_temp)
    return W
